# revision 1
# baseline (speedup 1.0000x reference)
"""Multi-task MoE routing (nn_CGC_69836168233304) on 8 TRN2 NeuronCores.

Reference math:
  h[g,e] = relu(x @ W[g,e] + b[g,e])                   12 experts (3 groups x 4)
  sel_t  = softmax(x @ Wg[t] + bg[t])   over 8 cols    t in {0,1}
  sel_s  = softmax(x @ Wgs + bgs)       over 12 cols
  out_t  = sum_m sel_t[:,m] * concat(h[t], h[2])[m]    t in {0,1}
  out_s  = sum_m sel_s[:,m] * concat(h[0],h[1],h[2])[m]

Sharding: data-parallel over batch B=16384 -> 2048 rows/core; every core holds
all 12 expert weights (streamed from HBM per O-slice) and produces its batch
shard of all three outputs; host concatenates shards (no collectives needed).

Per-core kernel (Bass/Tile), batch-major (batch rows on PSUM partitions):
  - K=1024 contraction split: chunks 0-5 in fp16 (full-rate matmuls), chunks
    6-7 in fp8e4m3 DoubleRow mode (one K=256 matmul at 2x PE rate). ALL
    expert weights (fp16 and fp8) are pre-scaled by 64 on host -- this lifts
    the fp8 W values out of e4m3's subnormal range while keeping PSUM
    uniform at 64*(x@W + b); the 1/64 rides the ACT scale (relu is
    positively homogeneous). CPU-simulated rel err ~1.8e-2 (< 2e-2 budget).
  - expert bias enters PSUM via a K=1 fp8 DoubleRow ones-matmul (256 beats
    instead of 512 for a fp16 ones-matmul).
  - gating: fused [D,28] fp16 gate matmul + segment softmax producing
    sel64 = sel/64. Since relu(s*z) = s*relu(z) for s>=0, the ACT pass that
    drains PSUM applies scale=sel64 per partition, emitting sel*h directly:
    2-contribution experts = 2 ACT passes, no vector multiply at all.
    3-contribution experts: one unscaled relu (64h) + three fast-mode
    tensor_scalar products (tmp * sel64) on DVE. Accumulate via fp16
    tensor_tensor adds on DVE (2x_1p). GpSimd/Pool is avoided
    (TensorScalarPtr unsupported there; software tensor ops are slow).
  - schedule: x is streamed per batch-tile ([128,8,128] slices, 2KB/partition
    lines) with one-tile-ahead prefetch; expert 8 (a 3-contribution expert
    whose PSUM-drain relu does not depend on sel) runs first with the gate
    matmul + softmax for each bt interleaved right before its matmuls, so
    the PE pipeline fills at ~4us instead of idling behind a monolithic x
    DMA + gate phase. The three per-bt accumulators live in one [128,3,512]
    tile, written out as a single DMA per (bt, osl) emitted inside the LAST
    expert's bt loop so the output DMAs overlap its compute instead of
    serializing after it (HWDGE issue is ~625ns per DMA).
  - outputs fp16 on device ([3, BC, O]), cast to fp32 on host.
"""

import numpy as np

import concourse.bacc as bacc
import concourse.mybir as mybir
import concourse.tile as tile
from concourse.bass_utils import run_bass_kernel_spmd

F32 = mybir.dt.float32
F16 = mybir.dt.float16
F8 = mybir.dt.float8e4

N_CORES = 8
B, D, O = 16384, 1024, 1024
BC = B // N_CORES
NE = 12  # experts
NG = 28  # gate columns: 8 (task0) + 8 (task1) + 12 (shared)
SEGS = ((0, 8), (8, 16), (16, 28))
OSL = 512  # output-column slice per PSUM bank
KB = 8  # 128-row K chunks in D
KF = 6  # chunks 0..KF-1 in fp16; chunks KF..7 in fp8 DoubleRow
WSCALE = 64.0  # weight pre-scale (fp16 and fp8), undone in the ACT scale
N_BT = BC // 128  # 16 batch tiles per core

# expert 8 first: it touches all three accumulators (first-touch init) and
# its PSUM-drain ACT needs no gate values, so the gate phase interleaves
# with it without stalling the ACT queue.
E_ORDER = [8, 9, 10, 11, 0, 1, 2, 3, 4, 5, 6, 7]


def _contribs(e):
    """(out_k, sel_col) pairs for expert e. Gate col order: t0=[g0e0..3,g2e0..3],
    t1=[g1e0..3,g2e0..3], shared=[g0,g1,g2]."""
    if e < 4:
        return [(0, e), (2, 16 + e)]
    if e < 8:
        return [(1, 8 + (e - 4)), (2, 20 + (e - 4))]
    return [(0, 4 + (e - 8)), (1, 12 + (e - 8)), (2, 24 + (e - 8))]


def _build(reps=1):
    N_OSL = O // OSL

    nc = bacc.Bacc("TRN2", target_bir_lowering=False, debug=False)

    # x pre-tiled per batch-tile with 2KB/partition contiguous lines:
    # xbt[bt, p, k, c] = x[bt*128+c, k*128+p]
    xbt_d = nc.dram_tensor("xbt", [N_BT, 128, KB, 128], F16, kind="ExternalInput")
    x8_d = nc.dram_tensor("x8bt", [N_BT, 128, 2, 128], F8, kind="ExternalInput")
    w16_d = nc.dram_tensor("We16", [NE, KF * 128, O], F16, kind="ExternalInput")
    w8_d = nc.dram_tensor("We8", [NE, 128, 2, O], F8, kind="ExternalInput")
    b8_d = nc.dram_tensor("Be8", [1, NE, 2, O], F8, kind="ExternalInput")
    ones8_d = nc.dram_tensor("ones8", [1, 2, 128], F8, kind="ExternalInput")
    wg_d = nc.dram_tensor("Wgc", [D, NG], F16, kind="ExternalInput")
    bg_d = nc.dram_tensor("bgc", [1, NG], F16, kind="ExternalInput")
    be_d = nc.dram_tensor("Be16", [1, NE, O], F16, kind="ExternalInput")
    out_d = nc.dram_tensor("outA", [3, BC, O], F16, kind="ExternalOutput")

    # experts whose bias rides a DVE tensor_tensor (PSUM + replicated bias)
    # instead of a PE ones-matmul: trades idle DVE time for PE beats, which
    # bound the kernel on real hardware. Only the 2-contribution experts:
    # extending this to the 3-contribution ones (tried) makes their per-tile
    # DVE work exceed the PE period, and the PSUM-drain backlog overruns the
    # 6-bank runway -- the sim shows ~50us of new PE stalls.
    DVE_BIAS = set(range(8))
    BSLOT = {e: i for i, e in enumerate(sorted(DVE_BIAS))}

    with tile.TileContext(nc) as tc:
        with (
            tc.tile_pool(name="big", bufs=1) as big,
            tc.tile_pool(name="wpool", bufs=2) as wpool,
            tc.tile_pool(name="accp", bufs=1) as accp,
            tc.tile_pool(name="tmpp", bufs=6) as tmpp,
            tc.tile_pool(name="gatep", bufs=2) as gatep,
            tc.tile_pool(name="psum", bufs=6, space="PSUM") as psum,
            tc.tile_pool(name="psumg", bufs=1, space="PSUM") as psumg,
        ):
            for rep in range(reps):
                # --- small resident staging ---
                wg_sb = big.tile([128, KB, NG], F16, tag="wg")
                nc.sync.dma_start(
                    wg_sb[:], wg_d.ap().rearrange("(k p) g -> p k g", p=128)
                )
                bg_sb = big.tile([1, NG], F16, tag="bg")
                nc.sync.dma_start(bg_sb[:], bg_d.ap())
                b8_sb = big.tile([1, NE, 2, O], F8, tag="b8")
                nc.sync.dma_start(b8_sb[:], b8_d.ap())
                ones8_sb = big.tile([1, 2, 128], F8, tag="ones8")
                nc.sync.dma_start(ones8_sb[:], ones8_d.ap())
                be_sb = big.tile([1, NE, O], F16, tag="be16")
                nc.sync.dma_start(be_sb[:], be_d.ap())
                ones_sb = big.tile([1, 128], F16, tag="ones")
                nc.vector.memset(ones_sb[:], 1.0)

                # x stream: first bt slice now, the rest prefetched one-ahead
                # from inside the first expert's bt loop.
                x_sb = big.tile([128, N_BT, KB, 128], F16, tag="x")
                x8_sb = big.tile([128, N_BT, 2, 128], F8, tag="x8")
                nc.sync.dma_start(x_sb[:, 0], xbt_d.ap()[0])
                nc.sync.dma_start(x8_sb[:, 0], x8_d.ap()[0])

                # sel64[:, bt, col] = softmax(...)[col] / WSCALE
                sel_sb = big.tile([128, N_BT, NG], F32, tag="sel")

                def emit_gate(bt):
                    pg = psumg.tile([128, NG], F32)
                    for k in range(KB):
                        nc.tensor.matmul(
                            pg[:], x_sb[:, bt, k, :], wg_sb[:, k, :],
                            start=(k == 0), stop=False,
                        )
                    nc.tensor.matmul(
                        pg[:], ones_sb[:], bg_sb[:], start=False, stop=True
                    )
                    et = gatep.tile([128, NG], F32)
                    nc.scalar.activation(
                        et[:], pg[:], mybir.ActivationFunctionType.Exp
                    )
                    for s0, s1 in SEGS:
                        den = gatep.tile([128, 1], F32, tag="den")
                        nc.vector.tensor_reduce(
                            den[:], et[:, s0:s1], mybir.AxisListType.X,
                            mybir.AluOpType.add,
                        )
                        den64 = gatep.tile([128, 1], F32, tag="den64")
                        nc.vector.tensor_scalar(
                            den64[:], den[:], WSCALE, None, mybir.AluOpType.mult
                        )
                        rden = gatep.tile([128, 1], F32, tag="rden")
                        nc.vector.reciprocal(rden[:], den64[:])
                        nc.vector.tensor_scalar(
                            sel_sb[:, bt, s0:s1], et[:, s0:s1], rden[:], None,
                            mybir.AluOpType.mult,
                        )

                # --- experts + gated accumulation ---
                for osl in range(N_OSL):
                    o0 = osl * OSL
                    # replicate 64*b across partitions for the DVE-bias
                    # experts: ones-matmul into PSUM + ACT copy to SBUF
                    brep = big.tile(
                        [128, len(DVE_BIAS), OSL], F32, tag=f"brep{osl}",
                        name="brep",
                    )
                    for e in sorted(DVE_BIAS):
                        psb = psum.tile(
                            [128, OSL], F32, tag="psb", name="psb", bufs=1
                        )
                        nc.tensor.matmul(
                            psb[:], ones_sb[:], be_sb[:, e, o0 : o0 + OSL],
                            start=True, stop=True,
                        )
                        nc.scalar.copy(brep[:, BSLOT[e], :], psb[:])
                    touched = set()
                    acct = {}
                    for ei, e in enumerate(E_ORDER):
                        first_e = ei == 0
                        last_e = ei == len(E_ORDER) - 1
                        w_sb = wpool.tile([128, KF, OSL], F16, tag="w16")
                        nc.sync.dma_start(
                            w_sb[:],
                            w16_d.ap()[e, :, o0 : o0 + OSL].rearrange(
                                "(k p) o -> p k o", p=128
                            ),
                        )
                        w8_sb = wpool.tile([128, 2, OSL], F8, tag="w8")
                        nc.sync.dma_start(
                            w8_sb[:], w8_d.ap()[e, :, :, o0 : o0 + OSL]
                        )
                        for bt in range(N_BT):
                            if first_e and osl == 0:
                                if bt + 1 < N_BT:  # prefetch next x slice
                                    nc.sync.dma_start(
                                        x_sb[:, bt + 1], xbt_d.ap()[bt + 1]
                                    )
                                    nc.sync.dma_start(
                                        x8_sb[:, bt + 1], x8_d.ap()[bt + 1]
                                    )
                                emit_gate(bt)
                            ps = psum.tile([128, OSL], F32)
                            for k in range(KF):
                                nc.tensor.matmul(
                                    ps[:],
                                    x_sb[:, bt, k, :],
                                    w_sb[:, k, :],
                                    start=(k == 0),
                                    stop=False,
                                )
                            dve_bias = e in DVE_BIAS
                            nc.tensor.matmul(
                                ps[:],
                                x8_sb[:, bt],
                                w8_sb[:],
                                start=False,
                                stop=dve_bias,
                                perf_mode=mybir.MatmulPerfMode.DoubleRow,
                            )
                            if not dve_bias:
                                nc.tensor.matmul(
                                    ps[:],
                                    ones8_sb[:],
                                    b8_sb[:, e, :, o0 : o0 + OSL],
                                    start=False,
                                    stop=True,
                                    perf_mode=mybir.MatmulPerfMode.DoubleRow,
                                )
                            src = ps[:]
                            if dve_bias:
                                # bias-add off the PE: tmp2 = ps + 64b (fp16)
                                tmp2 = tmpp.tile([128, OSL], F16, tag="tmp2")
                                nc.vector.tensor_tensor(
                                    tmp2[:], ps[:], brep[:, BSLOT[e], :],
                                    mybir.AluOpType.add,
                                )
                                src = tmp2[:]
                            if bt not in acct:
                                a = accp.tile(
                                    [128, 3, OSL], F16, tag=f"acc{bt}",
                                    name=f"acc{bt}",
                                )
                                acct[bt] = a
                            a = acct[bt]
                            prods = []  # (k, product tile) pending adds
                            if e < 8:
                                # sel*h straight out of ACT: relu(ps*sel64)
                                for k, col in _contribs(e):
                                    sc = sel_sb[:, bt, col : col + 1]
                                    if (k, bt) not in touched:
                                        touched.add((k, bt))
                                        dst = a[:, k, :]
                                    else:
                                        p = tmpp.tile(
                                            [128, OSL], F16, tag="prod"
                                        )
                                        prods.append((k, p))
                                        dst = p[:]
                                    nc.scalar.activation(
                                        dst, src,
                                        mybir.ActivationFunctionType.Relu,
                                        scale=sc,
                                    )
                            else:
                                # tmp = relu(ps) = 64h; products via fast
                                # tensor_scalar with sel64
                                tmp = tmpp.tile([128, OSL], F16, tag="tmp")
                                nc.scalar.activation(
                                    tmp[:], src,
                                    mybir.ActivationFunctionType.Relu,
                                )
                                for k, col in _contribs(e):
                                    sc = sel_sb[:, bt, col : col + 1]
                                    if (k, bt) not in touched:
                                        touched.add((k, bt))
                                        dst = a[:, k, :]
                                    else:
                                        p = tmpp.tile(
                                            [128, OSL], F16, tag="prod"
                                        )
                                        prods.append((k, p))
                                        dst = p[:]
                                    nc.vector.tensor_scalar(
                                        dst, tmp[:], sc, None,
                                        mybir.AluOpType.mult,
                                    )
                            for k, p in prods:
                                nc.vector.tensor_tensor(
                                    a[:, k, :], a[:, k, :], p[:],
                                    mybir.AluOpType.add,
                                )
                            if last_e:
                                nc.sync.dma_start(
                                    out_d.ap()[
                                        :, bt * 128 : (bt + 1) * 128,
                                        o0 : o0 + OSL,
                                    ].rearrange("k p o -> p k o"),
                                    a[:],
                                )

    nc.compile()
    return nc


_NC_CACHE = None


def get_nc():
    global _NC_CACHE
    if _NC_CACHE is None:
        _NC_CACHE = _build()
    return _NC_CACHE


def build_timing(reps):
    return _build(reps=reps)


def make_in_maps(inputs):
    x = np.asarray(inputs["x"], dtype=np.float32)
    W = np.asarray(inputs["W"], dtype=np.float32).reshape(NE, D, O)
    b = np.asarray(inputs["b"], dtype=np.float32).reshape(NE, O)
    Wg = np.asarray(inputs["Wg"], dtype=np.float32)
    bg = np.asarray(inputs["bg"], dtype=np.float32)
    Wgs = np.asarray(inputs["Wgs"], dtype=np.float32)
    bgs = np.asarray(inputs["bgs"], dtype=np.float32)

    import ml_dtypes

    f8 = ml_dtypes.float8_e4m3

    # fp16 chunks 0..KF-1, pre-scaled by WSCALE so PSUM is uniformly 64x
    w16 = np.ascontiguousarray(
        (W[:, : KF * 128, :] * WSCALE).astype(np.float16)
    )
    # fp8 chunks KF..7: [NE, 128, 2, O], 64*W
    w8 = np.ascontiguousarray(
        (W[:, KF * 128 :, :] * WSCALE)
        .reshape(NE, 2, 128, O)
        .transpose(0, 2, 1, 3)
    ).astype(f8)
    # bias: plane0 = 64*b, plane1 = 0
    b8 = np.zeros((1, NE, 2, O), np.float32)
    b8[0, :, 0, :] = b * WSCALE
    b8 = b8.astype(f8)
    ones8 = np.zeros((1, 2, 128), np.float32)
    ones8[0, 0, :] = 1.0
    ones8 = ones8.astype(f8)

    shared = {
        "We16": w16,
        "We8": w8,
        "Be8": b8,
        "Be16": (b * WSCALE)[None, :, :].astype(np.float16),
        "ones8": ones8,
        "Wgc": np.concatenate([Wg[0], Wg[1], Wgs], axis=1).astype(np.float16),
        "bgc": np.concatenate([bg[0], bg[1], bgs])[None, :].astype(np.float16),
    }
    in_maps = []
    for c in range(N_CORES):
        m = dict(shared)
        xc = x[c * BC : (c + 1) * BC]
        # xbt[bt, p, k, c] = x[bt*128+c, k*128+p]
        m["xbt"] = np.ascontiguousarray(
            xc.reshape(N_BT, 128, KB, 128).transpose(0, 3, 2, 1)
        ).astype(np.float16)
        m["x8bt"] = np.ascontiguousarray(
            xc[:, KF * 128 :]
            .reshape(N_BT, 128, 2, 128)
            .transpose(0, 3, 2, 1)
        ).astype(f8)
        in_maps.append(m)
    return in_maps


def kernel(x, W, b, Wg, bg, Wgs, bgs):
    nc = get_nc()
    in_maps = make_in_maps(
        {"x": x, "W": W, "b": b, "Wg": Wg, "bg": bg, "Wgs": Wgs, "bgs": bgs}
    )
    res = run_bass_kernel_spmd(nc, in_maps, list(range(N_CORES)))
    return tuple(
        np.concatenate(
            [res.results[c]["outA"][k] for c in range(N_CORES)], axis=0
        ).astype(np.float32)
        for k in range(3)
    )

